# revision 4
# baseline (speedup 1.0000x reference)
"""GATv2 layer (nn_GATv2Layer_12979391169461) Trainium2 Bass kernel.

Reference math (N=2048, F=128, HEADS=8, OUT_DIM=8, alpha=0.2):
    h  = (X @ W).reshape(N, 8, 8)
    s1 = h . a1   # [N, 8]
    s2 = h . a2   # [N, 8]
    e[n,j,k]   = lrelu(s1[n,k] + s2[j,k]) masked by A[n,j] (-1e9)
    att[n,j,k] = softmax_j(e[n,j,k])
    out[n,j,d] = sum_k att[n,j,k] * h[n,k,d]   # contracts the HEAD axis
    return lrelu(out).reshape(N*N/8, 64)

Key algebra used on device:
  * softmax over j is invariant to any per-(n,k) factor, so exp(s1) cancels:
      att numerator ~ m[n,j] * max(exp(s2[j,k]), r[n,k] * exp(0.2*s2[j,k]))
      with r = exp(-0.8*s1)   (uses exp(lrelu(x)) = max(exp x, exp 0.2x))
  * E2 = exp(s2), E2b = exp(0.2*s2) are per-j tables computed once per core,
    replicated over the 128 partitions (partition p = n_local*8 + head).
  * The 0/1 mask is replicated across heads by a tiny PE matmul (REPL16 @ A-rows)
    directly into PSUM, so no DMA bandwidth is spent on mask replication.
  * One fused DVE scalar_tensor_tensor computes q = v * mask AND the softmax
    denominator (accum_out) in a single pass.
  * The per-n [2048,8] @ [8,8] head-mix is batched 16 rows at a time as one
    block-diagonal [128,128] x [128,2048] matmul (1/denominator folded into
    the weights).
  * Final leaky-relu + PSUM->SBUF eviction is a single ACT Prelu pass.

Each of the 8 cores owns 256 rows (n) of the output. The device writes rows in
(n_block, n_local, d) x (j) order; the host transposes to the reference
(n, j, d) order while unsharding.
"""

import os
import sys
from contextlib import ExitStack

import numpy as np

sys.path.insert(0, "/opt/trn_rl_repo")

import concourse.bass as bass  # noqa: E402
import concourse.tile as tile  # noqa: E402
from concourse import bacc, mybir  # noqa: E402
from concourse.bass_utils import run_bass_kernel_spmd  # noqa: E402

N, F = 2048, 128
HEADS, OUT_DIM = 8, 8
ALPHA = 0.2
NCORES = 8
ROWS = N // NCORES          # 256 own rows per core
BLOCKS = ROWS // 16         # 16 blocks of 16 rows
FP = mybir.dt.float32
AOP = mybir.AluOpType

# score dtype: bf16 halves DVE time on the big elementwise passes
SCORE_BF16 = os.environ.get("GAT_SCORE_BF16", "1") == "1"
SDT = mybir.dt.bfloat16 if SCORE_BF16 else FP


def _mm_chunks(nc, out_ps, lhsT, rhs, free, maxn):
    """matmul out = lhsT.T @ rhs with the moving operand split into <=maxn cols."""
    for c0 in range(0, free, maxn):
        c1 = min(c0 + maxn, free)
        nc.tensor.matmul(out_ps[:, c0:c1], lhsT, rhs[:, c0:c1], start=True, stop=True)


def build_program():
    nc = bacc.Bacc("TRN2", debug=False)

    xt_d = nc.dram_tensor("XT", [F, N], FP, kind="ExternalInput")
    xto_d = nc.dram_tensor("XTo", [F, ROWS], FP, kind="ExternalInput")
    w_d = nc.dram_tensor("Wmat", [F, 64], FP, kind="ExternalInput")
    a1_d = nc.dram_tensor("A1BLK", [64, HEADS], FP, kind="ExternalInput")
    a2_d = nc.dram_tensor("A2BLK", [64, HEADS], FP, kind="ExternalInput")
    mask_d = nc.dram_tensor("MASKB", [ROWS, N], SDT, kind="ExternalInput")
    repl16_d = nc.dram_tensor("REPL16", [16, 128], SDT, kind="ExternalInput")
    repl8_d = nc.dram_tensor("REPL8", [HEADS, 128], FP, kind="ExternalInput")
    bd_d = nc.dram_tensor("BD_MASK", [128, 128], FP, kind="ExternalInput")
    id_d = nc.dram_tensor("IDENT", [128, 128], FP, kind="ExternalInput")
    out_d = nc.dram_tensor("OUTC", [ROWS * 8, N], FP, kind="ExternalOutput")

    MMF = 512   # fp32 moving-operand free-dim limit
    MMB = 1024  # bf16 limit

    with ExitStack() as ctx:
        tc = ctx.enter_context(tile.TileContext(nc))
        # persistent SBUF state
        per = ctx.enter_context(tc.tile_pool(name="persist", bufs=1))
        e2_rep = per.tile([128, N], SDT, tag="e2")
        e2b_rep = per.tile([128, N], SDT, tag="e2b")
        h_nmaj = [per.tile([128, 64], FP, tag=f"hn{i}", name=f"hn{i}") for i in range(2)]
        r_nmaj = [per.tile([128, HEADS], FP, tag=f"rn{i}", name=f"rn{i}") for i in range(2)]
        bd_mask = per.tile([128, 128], FP, tag="bd")
        alpha_v = per.tile([128, 1], FP, tag="al")
        nc.vector.memset(alpha_v[:], ALPHA)
        nc.gpsimd.dma_start(bd_mask[:], bd_d.ap())

        # ---------------- preprocessing ----------------
        with tc.tile_pool(name="pre", bufs=1) as pre, \
             tc.tile_pool(name="pre_ps", bufs=1, space="PSUM") as pre_ps:
            ident = pre.tile([128, 128], FP)
            nc.gpsimd.dma_start(ident[:], id_d.ap())
            wmat = pre.tile([F, 64], FP)
            nc.gpsimd.dma_start(wmat[:], w_d.ap())
            xt = pre.tile([F, N], FP)
            nc.gpsimd.dma_start(xt[:], xt_d.ap())
            xto = pre.tile([F, ROWS], FP)
            nc.gpsimd.dma_start(xto[:], xto_d.ap())
            a1b = pre.tile([64, HEADS], FP)
            nc.gpsimd.dma_start(a1b[:], a1_d.ap())
            a2b = pre.tile([64, HEADS], FP)
            nc.gpsimd.dma_start(a2b[:], a2_d.ap())
            repl8 = pre.tile([HEADS, 128], FP)
            nc.gpsimd.dma_start(repl8[:], repl8_d.ap())

            # hT = (X W)^T : [64, N] ; hTo = own-rows slice [64, ROWS]
            ht_ps = pre_ps.tile([64, N], FP, tag="big")
            _mm_chunks(nc, ht_ps, wmat[:], xt[:], N, MMF)
            ht = pre.tile([64, N], FP)
            nc.scalar.copy(ht[:], ht_ps[:])
            hto_ps = pre_ps.tile([64, ROWS], FP, tag="small")
            _mm_chunks(nc, hto_ps, wmat[:], xto[:], ROWS, MMF)
            hto = pre.tile([64, ROWS], FP)
            nc.scalar.copy(hto[:], hto_ps[:])

            # s2T over all nodes -> exp tables, replicated x16 on partitions
            s2t_ps = pre_ps.tile([HEADS, N], FP, tag="big")
            _mm_chunks(nc, s2t_ps, a2b[:], ht[:], N, MMF)
            e2_row = pre.tile([HEADS, N], FP)
            nc.scalar.activation(e2_row[:], s2t_ps[:], mybir.ActivationFunctionType.Exp)
            e2b_row = pre.tile([HEADS, N], FP)
            nc.scalar.activation(e2b_row[:], s2t_ps[:],
                                 mybir.ActivationFunctionType.Exp, scale=ALPHA)
            rep_ps = pre_ps.tile([128, N], FP, tag="big")
            _mm_chunks(nc, rep_ps, repl8[:], e2_row[:], N, MMF)
            nc.scalar.copy(e2_rep[:], rep_ps[:])
            rep2_ps = pre_ps.tile([128, N], FP, tag="big")
            _mm_chunks(nc, rep2_ps, repl8[:], e2b_row[:], N, MMF)
            nc.scalar.copy(e2b_rep[:], rep2_ps[:])

            # s1 of own rows -> r = exp(-0.8*s1) in n-major [128, 8] halves
            s1o_ps = pre_ps.tile([HEADS, ROWS], FP, tag="small")
            _mm_chunks(nc, s1o_ps, a1b[:], hto[:], ROWS, MMF)
            s1o = pre.tile([HEADS, ROWS], FP)
            nc.scalar.copy(s1o[:], s1o_ps[:])
            for i in range(2):
                tp = pre_ps.tile([128, HEADS], FP, tag="tiny")
                nc.tensor.transpose(tp[:], s1o[:, i * 128:(i + 1) * 128],
                                    ident[:HEADS, :HEADS])
                nc.scalar.activation(r_nmaj[i][:], tp[:],
                                     mybir.ActivationFunctionType.Exp, scale=-0.8)
                tp2 = pre_ps.tile([128, 64], FP, tag="tiny")
                nc.tensor.transpose(tp2[:], hto[:, i * 128:(i + 1) * 128],
                                    ident[:64, :64])
                nc.scalar.copy(h_nmaj[i][:], tp2[:])

        # ---------------- main loop over 16-row blocks ----------------
        repl16 = per.tile([16, 128], SDT, tag="repl16")
        nc.gpsimd.dma_start(repl16[:], repl16_d.ap())

        sb = ctx.enter_context(tc.tile_pool(name="blk", bufs=2))
        sb_q = ctx.enter_context(tc.tile_pool(name="blkq", bufs=2))
        ps_m = ctx.enter_context(tc.tile_pool(name="psm", bufs=1, space="PSUM"))
        ps_y = ctx.enter_context(tc.tile_pool(name="psy", bufs=1, space="PSUM"))

        for b in range(BLOCKS):
            half, row = divmod(b * 16, 128)

            # mask rows -> PE-replicated [128, N] in PSUM (p = n_local*8 + x)
            maskb = sb.tile([16, N], SDT, tag="maskb")
            nc.gpsimd.dma_start(maskb[:], mask_d.ap()[b * 16:(b + 1) * 16, :])
            m_rep = ps_m.tile([128, N], FP, tag="mrep")
            _mm_chunks(nc, m_rep, repl16[:], maskb[:], N, MMB if SCORE_BF16 else MMF)

            # r_b [128,1]: rows (16,8) of r_nmaj half -> partitions n_local*8+h
            rb = sb.tile([128, 1], FP, tag="rb")
            nc.gpsimd.dma_start(rb[:], r_nmaj[half][row:row + 16, :])

            # v = max(E2, r*E2b)  (one fused DVE op)
            v = sb.tile([128, N], SDT, tag="v")
            nc.vector.scalar_tensor_tensor(v[:], e2b_rep[:], rb[:], e2_rep[:],
                                           op0=AOP.mult, op1=AOP.max)

            # q = v * mask ; Dq = sum_j q   (one fused DVE op)
            q = sb_q.tile([128, N], FP, tag="q")
            dq = sb.tile([128, 1], FP, tag="dq")
            nc.vector.scalar_tensor_tensor(q[:], v[:], 1.0, m_rep[:],
                                           op0=AOP.mult, op1=AOP.mult, accum_out=dq[:])

            # W_blk[p=nh, f=n'd] = h_own[n,h*8+d]/Dq[nh] * blockdiag(n==n')
            rdq = sb.tile([128, 1], FP, tag="rdq")
            nc.vector.reciprocal(rdq[:], dq[:])
            hb = sb.tile([128, HEADS], FP, tag="hb")
            nc.gpsimd.dma_start(hb[:], h_nmaj[half][row:row + 16, :])
            wblk = sb.tile([128, 128], FP, tag="wblk")
            nc.vector.scalar_tensor_tensor(
                wblk[:].rearrange("p (o e) -> p o e", o=16),
                hb[:].rearrange("p (o e) -> p o e", o=1).broadcast_to([128, 16, HEADS]),
                rdq[:],
                bd_mask[:].rearrange("p (o e) -> p o e", o=16),
                op0=AOP.mult, op1=AOP.mult)

            # y[p=nd, j] = sum_h W_blk[nh, nd] q[nh, j] ; out = lrelu(y)
            y_ps = ps_y.tile([128, N], FP, tag="y")
            _mm_chunks(nc, y_ps, wblk[:], q[:], N, MMF)
            out_sb = sb_q.tile([128, N], FP, tag="out")
            nc.scalar.activation(out_sb[:], y_ps[:],
                                 mybir.ActivationFunctionType.Prelu, alpha=alpha_v[:])
            nc.gpsimd.dma_start(out_d.ap()[b * 128:(b + 1) * 128, :], out_sb[:])

    nc.compile()
    return nc


_NC_CACHE = None


def _get_program():
    global _NC_CACHE
    if _NC_CACHE is None:
        _NC_CACHE = build_program()
    return _NC_CACHE


def _host_inputs(X, A, W, attn_kernel):
    import ml_dtypes
    mdt = ml_dtypes.bfloat16 if SCORE_BF16 else np.float32

    XT = np.ascontiguousarray(X.T).astype(np.float32)
    a1 = attn_kernel[:OUT_DIM, 0].astype(np.float32)
    a2 = attn_kernel[OUT_DIM:, 0].astype(np.float32)
    # block-diag expansion: s1T[h, n] = sum_d a1[d] * hT[h*8+d, n]
    A1B = np.zeros((64, HEADS), np.float32)
    A2B = np.zeros((64, HEADS), np.float32)
    for h in range(HEADS):
        A1B[h * OUT_DIM:(h + 1) * OUT_DIM, h] = a1
        A2B[h * OUT_DIM:(h + 1) * OUT_DIM, h] = a2
    REPL16 = np.zeros((16, 128), np.float32)
    for nl in range(16):
        REPL16[nl, nl * 8:(nl + 1) * 8] = 1.0
    REPL8 = np.zeros((HEADS, 128), np.float32)
    for nl in range(16):
        REPL8[:, nl * 8:(nl + 1) * 8] = np.eye(HEADS, dtype=np.float32)
    BD = np.zeros((128, 128), np.float32)
    for nl in range(16):
        BD[nl * 8:(nl + 1) * 8, nl * 8:(nl + 1) * 8] = 1.0
    IDENT = np.eye(128, dtype=np.float32)

    Af = (A > 0).astype(np.float32)
    in_maps = []
    for c in range(NCORES):
        n0 = c * ROWS
        in_maps.append({
            "XT": XT,
            "XTo": np.ascontiguousarray(XT[:, n0:n0 + ROWS]),
            "Wmat": W.astype(np.float32),
            "A1BLK": A1B, "A2BLK": A2B,
            "MASKB": Af[n0:n0 + ROWS].astype(mdt),
            "REPL16": REPL16.astype(mdt),
            "REPL8": REPL8, "BD_MASK": BD, "IDENT": IDENT,
        })
    return in_maps


def kernel(X, A, W, attn_kernel, _want_timing=False):
    X = np.asarray(X)
    A = np.asarray(A)
    W = np.asarray(W)
    attn_kernel = np.asarray(attn_kernel)
    nc = _get_program()
    in_maps = _host_inputs(X, A, W, attn_kernel)
    res = run_bass_kernel_spmd(nc, in_maps, core_ids=list(range(NCORES)),
                               trace=_want_timing)
    # device rows are (block, n_local, d) x (j); reference wants (n, j, d)
    parts = []
    for c in range(NCORES):
        oc = res.results[c]["OUTC"]                        # [2048, 2048]
        oc = oc.reshape(BLOCKS, 16, OUT_DIM, N)            # [b, nl, d, j]
        oc = oc.transpose(0, 1, 3, 2).reshape(-1, OUT_DIM * HEADS)
        parts.append(oc)
    out = np.concatenate(parts, axis=0)
    if _want_timing:
        return out, res
    return out


# revision 7
# speedup vs baseline: 1.6945x; 1.6945x over previous
"""GATv2 layer (nn_GATv2Layer_12979391169461) Trainium2 Bass kernel.

Reference math (N=2048, F=128, HEADS=8, OUT_DIM=8, alpha=0.2):
    h  = (X @ W).reshape(N, 8, 8)
    s1 = h . a1   # [N, 8]
    s2 = h . a2   # [N, 8]
    e[n,j,k]   = lrelu(s1[n,k] + s2[j,k]) masked by A[n,j] (-1e9)
    att[n,j,k] = softmax_j(e[n,j,k])
    out[n,j,d] = sum_k att[n,j,k] * h[n,k,d]   # contracts the HEAD axis
    return lrelu(out).reshape(N*N/8, 64)

Key algebra used on device:
  * softmax over j is invariant to any per-(n,k) factor, so exp(s1) cancels:
      att numerator ~ m[n,j] * max(exp(s2[j,k]), r[n,k] * exp(0.2*s2[j,k]))
      with r = exp(-0.8*s1)   (uses exp(lrelu(x)) = max(exp x, exp 0.2x))
  * E2 = exp(s2), E2b = exp(0.2*s2) are per-j tables computed once per core,
    replicated over the 128 partitions (partition p = n_local*8 + head).
  * The 0/1 mask is replicated across heads by a tiny PE matmul (REPL16 @ A-rows)
    directly into PSUM, so no DMA bandwidth is spent on mask replication.
  * One fused DVE scalar_tensor_tensor computes q = v * mask AND the softmax
    denominator (accum_out) in a single pass.
  * The per-n [2048,8] @ [8,8] head-mix is batched 16 rows at a time as one
    block-diagonal [128,128] x [128,2048] matmul (1/denominator folded into
    the weights).
  * Final leaky-relu + PSUM->SBUF eviction is a single ACT Prelu pass.

Each of the 8 cores owns 256 rows (n) of the output. The device writes rows in
(n_block, n_local, d) x (j) order; the host transposes to the reference
(n, j, d) order while unsharding.
"""

import os
import sys
from contextlib import ExitStack

import numpy as np

sys.path.insert(0, "/opt/trn_rl_repo")

import concourse.bass as bass  # noqa: E402
import concourse.tile as tile  # noqa: E402
from concourse import bacc, mybir  # noqa: E402
from concourse.bass_utils import run_bass_kernel_spmd  # noqa: E402

N, F = 2048, 128
HEADS, OUT_DIM = 8, 8
ALPHA = 0.2
NCORES = 8
ROWS = N // NCORES          # 256 own rows per core
BLOCKS = ROWS // 16         # 16 blocks of 16 rows
FP = mybir.dt.float32
FR = mybir.dt.float32r
AOP = mybir.AluOpType

# score dtype: bf16 halves DVE time on the big elementwise passes
SCORE_BF16 = os.environ.get("GAT_SCORE_BF16", "1") == "1"
SDT = mybir.dt.bfloat16 if SCORE_BF16 else FP


def _mm_chunks(nc, out_ps, lhsT, rhs, free, maxn):
    """matmul out = lhsT.T @ rhs with the moving operand split into <=maxn cols."""
    for c0 in range(0, free, maxn):
        c1 = min(c0 + maxn, free)
        nc.tensor.matmul(out_ps[:, c0:c1], lhsT, rhs[:, c0:c1], start=True, stop=True)


def build_program():
    nc = bacc.Bacc("TRN2", debug=False)

    xt_d = nc.dram_tensor("XT", [F, N], FP, kind="ExternalInput")
    xto_d = nc.dram_tensor("XTo", [F, ROWS], FP, kind="ExternalInput")
    w_d = nc.dram_tensor("Wmat", [F, 64], FP, kind="ExternalInput")
    a1_d = nc.dram_tensor("A1BLK", [64, HEADS], FP, kind="ExternalInput")
    a2_d = nc.dram_tensor("A2BLK", [64, HEADS], FP, kind="ExternalInput")
    mask_d = nc.dram_tensor("MASKB", [ROWS, N], SDT, kind="ExternalInput")
    repl16_d = nc.dram_tensor("REPL16", [128, 128], SDT, kind="ExternalInput")
    bd_d = nc.dram_tensor("BD_MASK", [128, 128], FP, kind="ExternalInput")
    id_d = nc.dram_tensor("IDENT", [128, 128], FP, kind="ExternalInput")
    out_d = nc.dram_tensor("OUTC", [ROWS * 8, N], FP, kind="ExternalOutput")

    MMF = 512   # fp32 moving-operand free-dim limit
    MMB = 512   # PSUM fp32 bank limit applies to output cols

    with ExitStack() as ctx:
        tc = ctx.enter_context(tile.TileContext(nc))
        # persistent SBUF state
        per = ctx.enter_context(tc.tile_pool(name="persist", bufs=1))
        e2_rep = per.tile([128, N], SDT, tag="e2")
        e2b_rep = per.tile([128, N], SDT, tag="e2b")
        h_nmaj = [per.tile([128, 64], FP, tag=f"hn{i}", name=f"hn{i}") for i in range(2)]
        r_nmaj = [per.tile([128, HEADS], FP, tag=f"rn{i}", name=f"rn{i}") for i in range(2)]
        bd_mask = per.tile([128, 128], FP, tag="bd")
        alpha_v = per.tile([128, 1], FP, tag="al")
        nc.vector.memset(alpha_v[:], ALPHA)
        nc.gpsimd.dma_start(bd_mask[:], bd_d.ap())

        # ---------------- preprocessing ----------------
        with tc.tile_pool(name="pre", bufs=1) as pre, \
             tc.tile_pool(name="pre_ps", bufs=1, space="PSUM") as pre_ps:
            ident = pre.tile([128, 128], FP)
            nc.gpsimd.dma_start(ident[:], id_d.ap())
            wmat = pre.tile([F, 64], FP)
            nc.gpsimd.dma_start(wmat[:], w_d.ap())
            xt = pre.tile([F, N], FP)
            nc.gpsimd.dma_start(xt[:], xt_d.ap())
            xto = pre.tile([F, ROWS], FP)
            nc.gpsimd.dma_start(xto[:], xto_d.ap())
            a1b = pre.tile([64, HEADS], FP)
            nc.gpsimd.dma_start(a1b[:], a1_d.ap())
            a2b = pre.tile([64, HEADS], FP)
            nc.gpsimd.dma_start(a2b[:], a2_d.ap())

            # hT = (X W)^T : [64, N] ; hTo = own-rows slice [64, ROWS]
            ht_ps = pre_ps.tile([64, N], FP, tag="big")
            _mm_chunks(nc, ht_ps, wmat[:], xt[:], N, MMF)
            ht = pre.tile([64, N], FP)
            nc.scalar.copy(ht[:], ht_ps[:])
            hto_ps = pre_ps.tile([64, ROWS], FP, tag="small")
            _mm_chunks(nc, hto_ps, wmat[:], xto[:], ROWS, MMF)
            hto = pre.tile([64, ROWS], FP)
            nc.scalar.copy(hto[:], hto_ps[:])

            # s2T over all nodes -> exp tables, replicated x16 on partitions
            s2t_ps = pre_ps.tile([HEADS, N], FP, tag="big")
            _mm_chunks(nc, s2t_ps, a2b[:], ht[:], N, MMF)
            nc.scalar.activation(e2_rep[:HEADS, :], s2t_ps[:],
                                 mybir.ActivationFunctionType.Exp)
            nc.scalar.activation(e2b_rep[:HEADS, :], s2t_ps[:],
                                 mybir.ActivationFunctionType.Exp, scale=ALPHA)
            for nl in range(1, 16):
                nc.gpsimd.dma_start(e2_rep[nl * 8:(nl + 1) * 8, :], e2_rep[:HEADS, :])
                nc.gpsimd.dma_start(e2b_rep[nl * 8:(nl + 1) * 8, :], e2b_rep[:HEADS, :])

            # s1 of own rows -> r = exp(-0.8*s1) in n-major [128, 8] halves
            s1o_ps = pre_ps.tile([HEADS, ROWS], FP, tag="small")
            _mm_chunks(nc, s1o_ps, a1b[:], hto[:], ROWS, MMF)
            s1o = pre.tile([HEADS, ROWS], FP)
            nc.scalar.copy(s1o[:], s1o_ps[:])
            for i in range(2):
                tp = pre_ps.tile([128, HEADS], FP, tag="tiny")
                nc.tensor.transpose(tp[:], s1o[:, i * 128:(i + 1) * 128],
                                    ident[:HEADS, :HEADS])
                nc.scalar.activation(r_nmaj[i][:], tp[:],
                                     mybir.ActivationFunctionType.Exp, scale=-0.8)
                tp2 = pre_ps.tile([128, 64], FP, tag="tiny")
                nc.tensor.transpose(tp2[:], hto[:, i * 128:(i + 1) * 128],
                                    ident[:64, :64])
                nc.scalar.copy(h_nmaj[i][:], tp2[:])

        # ---------------- main loop over 16-row blocks ----------------
        repl16 = per.tile([128, 128], SDT, tag="repl16")
        nc.gpsimd.dma_start(repl16[:], repl16_d.ap())
        # manual double-buffered padded mask tiles (rows 16+ stay zero)
        maskp = [per.tile([128, N], SDT, tag=f"maskp{i}", name=f"maskp{i}")
                 for i in range(2)]
        nc.vector.memset(maskp[0][:], 0.0)
        nc.vector.memset(maskp[1][:], 0.0)

        sb = ctx.enter_context(tc.tile_pool(name="blk", bufs=2))
        sb_q = ctx.enter_context(tc.tile_pool(name="blkq", bufs=2))
        ps_m = ctx.enter_context(tc.tile_pool(name="psm", bufs=1, space="PSUM"))
        ps_y = ctx.enter_context(tc.tile_pool(name="psy", bufs=1, space="PSUM"))

        for b in range(BLOCKS):
            half, row = divmod(b * 16, 128)

            # mask rows -> PE-replicated [128, N] in PSUM (p = n_local*8 + x)
            maskb = maskp[b % 2]
            nc.gpsimd.dma_start(maskb[:16, :], mask_d.ap()[b * 16:(b + 1) * 16, :])
            m_rep = ps_m.tile([128, N], FP, tag="mrep")
            _mm_chunks(nc, m_rep, repl16[:], maskb[:], N, MMB if SCORE_BF16 else MMF)

            # r_b [128,1]: rows (16,8) of r_nmaj half -> partitions n_local*8+h
            rb = sb.tile([128, 1], FP, tag="rb")
            nc.gpsimd.dma_start(rb[:], r_nmaj[half][row:row + 16, :])

            # v = max(E2, r*E2b)  (one fused DVE op)
            v = sb.tile([128, N], SDT, tag="v")
            nc.vector.scalar_tensor_tensor(v[:], e2b_rep[:], rb[:], e2_rep[:],
                                           op0=AOP.mult, op1=AOP.max)

            # q = v * mask ; Dq = sum_j q   (one fused DVE op)
            q = sb_q.tile([128, N], FR, tag="q")
            dq = sb.tile([128, 1], FP, tag="dq")
            nc.vector.scalar_tensor_tensor(q[:], v[:], 1.0, m_rep[:],
                                           op0=AOP.mult, op1=AOP.mult, accum_out=dq[:])

            # W_blk[p=nh, f=n'd] = h_own[n,h*8+d]/Dq[nh] * blockdiag(n==n')
            rdq = sb.tile([128, 1], FP, tag="rdq")
            nc.vector.reciprocal(rdq[:], dq[:])
            hb = sb.tile([128, HEADS], FP, tag="hb")
            nc.gpsimd.dma_start(hb[:], h_nmaj[half][row:row + 16, :])
            wblk = sb.tile([128, 128], FR, tag="wblk")
            nc.vector.scalar_tensor_tensor(
                wblk[:].rearrange("p (o e) -> p o e", o=16),
                hb[:].rearrange("p (o e) -> p o e", o=1).broadcast_to([128, 16, HEADS]),
                rdq[:],
                bd_mask[:].rearrange("p (o e) -> p o e", o=16),
                op0=AOP.mult, op1=AOP.mult)

            # y[p=nd, j] = sum_h W_blk[nh, nd] q[nh, j] ; out = lrelu(y)
            y_ps = ps_y.tile([128, N], FP, tag="y")
            _mm_chunks(nc, y_ps, wblk[:], q[:], N, MMF)
            out_sb = sb_q.tile([128, N], FP, tag="out")
            nc.scalar.activation(out_sb[:], y_ps[:],
                                 mybir.ActivationFunctionType.Prelu, alpha=alpha_v[:])
            nc.gpsimd.dma_start(out_d.ap()[b * 128:(b + 1) * 128, :], out_sb[:])

    nc.compile()
    return nc


_NC_CACHE = None


def _get_program():
    global _NC_CACHE
    if _NC_CACHE is None:
        _NC_CACHE = build_program()
    return _NC_CACHE


def _host_inputs(X, A, W, attn_kernel):
    import ml_dtypes
    mdt = ml_dtypes.bfloat16 if SCORE_BF16 else np.float32

    XT = np.ascontiguousarray(X.T).astype(np.float32)
    a1 = attn_kernel[:OUT_DIM, 0].astype(np.float32)
    a2 = attn_kernel[OUT_DIM:, 0].astype(np.float32)
    # block-diag expansion: s1T[h, n] = sum_d a1[d] * hT[h*8+d, n]
    A1B = np.zeros((64, HEADS), np.float32)
    A2B = np.zeros((64, HEADS), np.float32)
    for h in range(HEADS):
        A1B[h * OUT_DIM:(h + 1) * OUT_DIM, h] = a1
        A2B[h * OUT_DIM:(h + 1) * OUT_DIM, h] = a2
    REPL16 = np.zeros((128, 128), np.float32)
    for nl in range(16):
        REPL16[nl, nl * 8:(nl + 1) * 8] = 1.0
    BD = np.zeros((128, 128), np.float32)
    for nl in range(16):
        BD[nl * 8:(nl + 1) * 8, nl * 8:(nl + 1) * 8] = 1.0
    IDENT = np.eye(128, dtype=np.float32)

    Af = (A > 0).astype(np.float32)
    in_maps = []
    for c in range(NCORES):
        n0 = c * ROWS
        in_maps.append({
            "XT": XT,
            "XTo": np.ascontiguousarray(XT[:, n0:n0 + ROWS]),
            "Wmat": W.astype(np.float32),
            "A1BLK": A1B, "A2BLK": A2B,
            "MASKB": Af[n0:n0 + ROWS].astype(mdt),
            "REPL16": REPL16.astype(mdt),
            "BD_MASK": BD, "IDENT": IDENT,
        })
    return in_maps


def kernel(X, A, W, attn_kernel, _want_timing=False):
    X = np.asarray(X)
    A = np.asarray(A)
    W = np.asarray(W)
    attn_kernel = np.asarray(attn_kernel)
    nc = _get_program()
    in_maps = _host_inputs(X, A, W, attn_kernel)
    res = run_bass_kernel_spmd(nc, in_maps, core_ids=list(range(NCORES)),
                               trace=_want_timing)
    # device rows are (block, n_local, d) x (j); reference wants (n, j, d)
    parts = []
    for c in range(NCORES):
        oc = res.results[c]["OUTC"]                        # [2048, 2048]
        oc = oc.reshape(BLOCKS, 16, OUT_DIM, N)            # [b, nl, d, j]
        oc = oc.transpose(0, 1, 3, 2).reshape(-1, OUT_DIM * HEADS)
        parts.append(oc)
    out = np.concatenate(parts, axis=0)
    if _want_timing:
        return out, res
    return out


# revision 8
# speedup vs baseline: 1.7626x; 1.0402x over previous
"""GATv2 layer (nn_GATv2Layer_12979391169461) Trainium2 Bass kernel.

Reference math (N=2048, F=128, HEADS=8, OUT_DIM=8, alpha=0.2):
    h  = (X @ W).reshape(N, 8, 8)
    s1 = h . a1   # [N, 8]
    s2 = h . a2   # [N, 8]
    e[n,j,k]   = lrelu(s1[n,k] + s2[j,k]) masked by A[n,j] (-1e9)
    att[n,j,k] = softmax_j(e[n,j,k])
    out[n,j,d] = sum_k att[n,j,k] * h[n,k,d]   # contracts the HEAD axis
    return lrelu(out).reshape(N*N/8, 64)

Key algebra used on device:
  * softmax over j is invariant to any per-(n,k) factor, so exp(s1) cancels:
      att numerator ~ m[n,j] * max(exp(s2[j,k]), r[n,k] * exp(0.2*s2[j,k]))
      with r = exp(-0.8*s1)   (uses exp(lrelu(x)) = max(exp x, exp 0.2x))
  * E2 = exp(s2), E2b = exp(0.2*s2) are per-j tables computed once per core,
    replicated over the 128 partitions (partition p = n_local*8 + head).
  * The 0/1 mask is replicated across heads by a tiny PE matmul (REPL16 @ A-rows)
    directly into PSUM, so no DMA bandwidth is spent on mask replication.
  * One fused DVE scalar_tensor_tensor computes q = v * mask AND the softmax
    denominator (accum_out) in a single pass.
  * The per-n [2048,8] @ [8,8] head-mix is batched 16 rows at a time as one
    block-diagonal [128,128] x [128,2048] matmul (1/denominator folded into
    the weights).
  * Final leaky-relu + PSUM->SBUF eviction is a single ACT Prelu pass.

Each of the 8 cores owns 256 rows (n) of the output. The device writes rows in
(n_block, n_local, d) x (j) order; the host transposes to the reference
(n, j, d) order while unsharding.
"""

import os
import sys
from contextlib import ExitStack

import numpy as np

sys.path.insert(0, "/opt/trn_rl_repo")

import concourse.bass as bass  # noqa: E402
import concourse.tile as tile  # noqa: E402
from concourse import bacc, mybir  # noqa: E402
from concourse.bass_utils import run_bass_kernel_spmd  # noqa: E402

N, F = 2048, 128
HEADS, OUT_DIM = 8, 8
ALPHA = 0.2
NCORES = 8
ROWS = N // NCORES          # 256 own rows per core
BLOCKS = ROWS // 16         # 16 blocks of 16 rows
FP = mybir.dt.float32
FR = mybir.dt.float32r
AOP = mybir.AluOpType

# score dtype: bf16 halves DVE time on the big elementwise passes
SCORE_BF16 = os.environ.get("GAT_SCORE_BF16", "1") == "1"
SDT = mybir.dt.bfloat16 if SCORE_BF16 else FP


def _mm_chunks(nc, out_ps, lhsT, rhs, free, maxn):
    """matmul out = lhsT.T @ rhs with the moving operand split into <=maxn cols."""
    for c0 in range(0, free, maxn):
        c1 = min(c0 + maxn, free)
        nc.tensor.matmul(out_ps[:, c0:c1], lhsT, rhs[:, c0:c1], start=True, stop=True)


def build_program():
    nc = bacc.Bacc("TRN2", debug=False)

    xt_d = nc.dram_tensor("XT", [F, N], FP, kind="ExternalInput")
    xto_d = nc.dram_tensor("XTo", [F, ROWS], FP, kind="ExternalInput")
    w_d = nc.dram_tensor("Wmat", [F, 64], FP, kind="ExternalInput")
    a1_d = nc.dram_tensor("A1BLK", [64, HEADS], FP, kind="ExternalInput")
    a2_d = nc.dram_tensor("A2BLK", [64, HEADS], FP, kind="ExternalInput")
    mask_d = nc.dram_tensor("MASKB", [ROWS, N], SDT, kind="ExternalInput")
    repl16_d = nc.dram_tensor("REPL16", [128, 128], SDT, kind="ExternalInput")
    bd_d = nc.dram_tensor("BD_MASK", [128, 128], FP, kind="ExternalInput")
    id_d = nc.dram_tensor("IDENT", [128, 128], FP, kind="ExternalInput")
    out_d = nc.dram_tensor("OUTC", [ROWS * 8, N], FP, kind="ExternalOutput")

    MMF = 512   # fp32 moving-operand free-dim limit
    MMB = 512   # PSUM fp32 bank limit applies to output cols

    with ExitStack() as ctx:
        tc = ctx.enter_context(tile.TileContext(nc))
        # persistent SBUF state
        per = ctx.enter_context(tc.tile_pool(name="persist", bufs=1))
        e2_rep = per.tile([128, N], SDT, tag="e2")
        s2t_rep = per.tile([128, N], SDT, tag="s2t")
        h_nmaj = [per.tile([128, 64], FP, tag=f"hn{i}", name=f"hn{i}") for i in range(2)]
        r_nmaj = [per.tile([128, HEADS], FP, tag=f"rn{i}", name=f"rn{i}") for i in range(2)]
        bd_mask = per.tile([128, 128], FP, tag="bd")
        alpha_v = per.tile([128, 1], FP, tag="al")
        nc.vector.memset(alpha_v[:], ALPHA)
        nc.gpsimd.dma_start(bd_mask[:], bd_d.ap())

        # ---------------- preprocessing ----------------
        with tc.tile_pool(name="pre", bufs=1) as pre, \
             tc.tile_pool(name="pre_ps", bufs=1, space="PSUM") as pre_ps:
            ident = pre.tile([128, 128], FP)
            nc.gpsimd.dma_start(ident[:], id_d.ap())
            wmat = pre.tile([F, 64], FP)
            nc.gpsimd.dma_start(wmat[:], w_d.ap())
            xt = pre.tile([F, N], FP)
            nc.gpsimd.dma_start(xt[:], xt_d.ap())
            xto = pre.tile([F, ROWS], FP)
            nc.gpsimd.dma_start(xto[:], xto_d.ap())
            a1b = pre.tile([64, HEADS], FP)
            nc.gpsimd.dma_start(a1b[:], a1_d.ap())
            a2b = pre.tile([64, HEADS], FP)
            nc.gpsimd.dma_start(a2b[:], a2_d.ap())

            # hT = (X W)^T : [64, N] ; hTo = own-rows slice [64, ROWS]
            ht_ps = pre_ps.tile([64, N], FP, tag="big")
            _mm_chunks(nc, ht_ps, wmat[:], xt[:], N, MMF)
            ht = pre.tile([64, N], FP)
            nc.scalar.copy(ht[:], ht_ps[:])
            hto_ps = pre_ps.tile([64, ROWS], FP, tag="small")
            _mm_chunks(nc, hto_ps, wmat[:], xto[:], ROWS, MMF)
            hto = pre.tile([64, ROWS], FP)
            nc.scalar.copy(hto[:], hto_ps[:])

            # s2T over all nodes -> exp tables, replicated x16 on partitions
            s2t_ps = pre_ps.tile([HEADS, N], FP, tag="big")
            _mm_chunks(nc, s2t_ps, a2b[:], ht[:], N, MMF)
            nc.scalar.activation(e2_rep[:HEADS, :], s2t_ps[:],
                                 mybir.ActivationFunctionType.Exp)
            nc.scalar.copy(s2t_rep[:HEADS, :], s2t_ps[:])
            for nl in range(1, 16):
                nc.gpsimd.dma_start(e2_rep[nl * 8:(nl + 1) * 8, :], e2_rep[:HEADS, :])
                nc.gpsimd.dma_start(s2t_rep[nl * 8:(nl + 1) * 8, :], s2t_rep[:HEADS, :])

            # s1 of own rows -> r = exp(-0.8*s1) in n-major [128, 8] halves
            s1o_ps = pre_ps.tile([HEADS, ROWS], FP, tag="small")
            _mm_chunks(nc, s1o_ps, a1b[:], hto[:], ROWS, MMF)
            s1o = pre.tile([HEADS, ROWS], FP)
            nc.scalar.copy(s1o[:], s1o_ps[:])
            for i in range(2):
                tp = pre_ps.tile([128, HEADS], FP, tag="tiny")
                nc.tensor.transpose(tp[:], s1o[:, i * 128:(i + 1) * 128],
                                    ident[:HEADS, :HEADS])
                nc.scalar.activation(r_nmaj[i][:], tp[:],
                                     mybir.ActivationFunctionType.Copy, scale=-0.8)
                tp2 = pre_ps.tile([128, 64], FP, tag="tiny")
                nc.tensor.transpose(tp2[:], hto[:, i * 128:(i + 1) * 128],
                                    ident[:64, :64])
                nc.scalar.copy(h_nmaj[i][:], tp2[:])

        # ---------------- main loop over 16-row blocks ----------------
        repl16 = per.tile([128, 128], SDT, tag="repl16")
        nc.gpsimd.dma_start(repl16[:], repl16_d.ap())
        # manual double-buffered padded mask tiles (rows 16+ stay zero)
        maskp = [per.tile([128, N], SDT, tag=f"maskp{i}", name=f"maskp{i}")
                 for i in range(2)]
        nc.vector.memset(maskp[0][:], 0.0)
        nc.vector.memset(maskp[1][:], 0.0)

        sb = ctx.enter_context(tc.tile_pool(name="blk", bufs=2))
        sb_q = ctx.enter_context(tc.tile_pool(name="blkq", bufs=2))
        ps_m = ctx.enter_context(tc.tile_pool(name="psm", bufs=1, space="PSUM"))
        ps_y = ctx.enter_context(tc.tile_pool(name="psy", bufs=1, space="PSUM"))

        for b in range(BLOCKS):
            half, row = divmod(b * 16, 128)

            # mask rows -> PE-replicated [128, N] in PSUM (p = n_local*8 + x)
            maskb = maskp[b % 2]
            nc.gpsimd.dma_start(maskb[:16, :], mask_d.ap()[b * 16:(b + 1) * 16, :])
            m_rep = ps_m.tile([128, N], FP, tag="mrep")
            _mm_chunks(nc, m_rep, repl16[:], maskb[:], N, MMB if SCORE_BF16 else MMF)

            # r_b [128,1]: rows (16,8) of r_nmaj half -> partitions n_local*8+h
            rb = sb.tile([128, 1], FP, tag="rb")
            nc.gpsimd.dma_start(rb[:], r_nmaj[half][row:row + 16, :])

            # u = r*E2b = exp(0.2*s2 - 0.8*s1)  on ACT; v = max(E2, u) on DVE
            u = sb.tile([128, N], SDT, tag="u")
            nc.scalar.activation(u[:], s2t_rep[:], mybir.ActivationFunctionType.Exp,
                                 bias=rb[:], scale=ALPHA)
            v = sb.tile([128, N], SDT, tag="v")
            nc.vector.tensor_tensor(v[:], u[:], e2_rep[:], AOP.max)

            # q = v * mask ; Dq = sum_j q   (one fused DVE op)
            q = sb_q.tile([128, N], FR, tag="q")
            dq = sb.tile([128, 1], FP, tag="dq")
            nc.vector.scalar_tensor_tensor(q[:], v[:], 1.0, m_rep[:],
                                           op0=AOP.mult, op1=AOP.mult, accum_out=dq[:])

            # W_blk[p=nh, f=n'd] = h_own[n,h*8+d]/Dq[nh] * blockdiag(n==n')
            rdq = sb.tile([128, 1], FP, tag="rdq")
            nc.vector.reciprocal(rdq[:], dq[:])
            hb = sb.tile([128, HEADS], FP, tag="hb")
            nc.gpsimd.dma_start(hb[:], h_nmaj[half][row:row + 16, :])
            wblk = sb.tile([128, 128], FR, tag="wblk")
            nc.vector.scalar_tensor_tensor(
                wblk[:].rearrange("p (o e) -> p o e", o=16),
                hb[:].rearrange("p (o e) -> p o e", o=1).broadcast_to([128, 16, HEADS]),
                rdq[:],
                bd_mask[:].rearrange("p (o e) -> p o e", o=16),
                op0=AOP.mult, op1=AOP.mult)

            # y[p=nd, j] = sum_h W_blk[nh, nd] q[nh, j] ; out = lrelu(y)
            y_ps = ps_y.tile([128, N], FP, tag="y")
            _mm_chunks(nc, y_ps, wblk[:], q[:], N, MMF)
            out_sb = sb_q.tile([128, N], FP, tag="out")
            nc.scalar.activation(out_sb[:], y_ps[:],
                                 mybir.ActivationFunctionType.Prelu, alpha=alpha_v[:])
            nc.gpsimd.dma_start(out_d.ap()[b * 128:(b + 1) * 128, :], out_sb[:])

    nc.compile()
    return nc


_NC_CACHE = None


def _get_program():
    global _NC_CACHE
    if _NC_CACHE is None:
        _NC_CACHE = build_program()
    return _NC_CACHE


def _host_inputs(X, A, W, attn_kernel):
    import ml_dtypes
    mdt = ml_dtypes.bfloat16 if SCORE_BF16 else np.float32

    XT = np.ascontiguousarray(X.T).astype(np.float32)
    a1 = attn_kernel[:OUT_DIM, 0].astype(np.float32)
    a2 = attn_kernel[OUT_DIM:, 0].astype(np.float32)
    # block-diag expansion: s1T[h, n] = sum_d a1[d] * hT[h*8+d, n]
    A1B = np.zeros((64, HEADS), np.float32)
    A2B = np.zeros((64, HEADS), np.float32)
    for h in range(HEADS):
        A1B[h * OUT_DIM:(h + 1) * OUT_DIM, h] = a1
        A2B[h * OUT_DIM:(h + 1) * OUT_DIM, h] = a2
    REPL16 = np.zeros((128, 128), np.float32)
    for nl in range(16):
        REPL16[nl, nl * 8:(nl + 1) * 8] = 1.0
    BD = np.zeros((128, 128), np.float32)
    for nl in range(16):
        BD[nl * 8:(nl + 1) * 8, nl * 8:(nl + 1) * 8] = 1.0
    IDENT = np.eye(128, dtype=np.float32)

    Af = (A > 0).astype(np.float32)
    in_maps = []
    for c in range(NCORES):
        n0 = c * ROWS
        in_maps.append({
            "XT": XT,
            "XTo": np.ascontiguousarray(XT[:, n0:n0 + ROWS]),
            "Wmat": W.astype(np.float32),
            "A1BLK": A1B, "A2BLK": A2B,
            "MASKB": Af[n0:n0 + ROWS].astype(mdt),
            "REPL16": REPL16.astype(mdt),
            "BD_MASK": BD, "IDENT": IDENT,
        })
    return in_maps


def kernel(X, A, W, attn_kernel, _want_timing=False):
    X = np.asarray(X)
    A = np.asarray(A)
    W = np.asarray(W)
    attn_kernel = np.asarray(attn_kernel)
    nc = _get_program()
    in_maps = _host_inputs(X, A, W, attn_kernel)
    res = run_bass_kernel_spmd(nc, in_maps, core_ids=list(range(NCORES)),
                               trace=_want_timing)
    # device rows are (block, n_local, d) x (j); reference wants (n, j, d)
    parts = []
    for c in range(NCORES):
        oc = res.results[c]["OUTC"]                        # [2048, 2048]
        oc = oc.reshape(BLOCKS, 16, OUT_DIM, N)            # [b, nl, d, j]
        oc = oc.transpose(0, 1, 3, 2).reshape(-1, OUT_DIM * HEADS)
        parts.append(oc)
    out = np.concatenate(parts, axis=0)
    if _want_timing:
        return out, res
    return out


# revision 9
# speedup vs baseline: 2.1005x; 1.1917x over previous
"""GATv2 layer (nn_GATv2Layer_12979391169461) Trainium2 Bass kernel.

Reference math (N=2048, F=128, HEADS=8, OUT_DIM=8, alpha=0.2):
    h  = (X @ W).reshape(N, 8, 8)
    s1 = h . a1   # [N, 8]
    s2 = h . a2   # [N, 8]
    e[n,j,k]   = lrelu(s1[n,k] + s2[j,k]) masked by A[n,j] (-1e9)
    att[n,j,k] = softmax_j(e[n,j,k])
    out[n,j,d] = sum_k att[n,j,k] * h[n,k,d]   # contracts the HEAD axis
    return lrelu(out).reshape(N*N/8, 64)

Key algebra used on device:
  * softmax over j is invariant to any per-(n,k) factor, so exp(s1) cancels:
      att numerator ~ m[n,j] * max(exp(s2[j,k]), r[n,k] * exp(0.2*s2[j,k]))
      with r = exp(-0.8*s1)   (uses exp(lrelu(x)) = max(exp x, exp 0.2x))
  * E2 = exp(s2), E2b = exp(0.2*s2) are per-j tables computed once per core,
    replicated over the 128 partitions (partition p = n_local*8 + head).
  * The 0/1 mask is replicated across heads by a tiny PE matmul (REPL16 @ A-rows)
    directly into PSUM, so no DMA bandwidth is spent on mask replication.
  * One fused DVE scalar_tensor_tensor computes q = v * mask AND the softmax
    denominator (accum_out) in a single pass.
  * The per-n [2048,8] @ [8,8] head-mix is batched 16 rows at a time as one
    block-diagonal [128,128] x [128,2048] matmul (1/denominator folded into
    the weights).
  * Final leaky-relu + PSUM->SBUF eviction is a single ACT Prelu pass.

Each of the 8 cores owns 256 rows (n) of the output. The device writes rows in
(n_block, n_local, d) x (j) order; the host transposes to the reference
(n, j, d) order while unsharding.
"""

import os
import sys
from contextlib import ExitStack

import numpy as np

sys.path.insert(0, "/opt/trn_rl_repo")

import concourse.bass as bass  # noqa: E402
import concourse.tile as tile  # noqa: E402
from concourse import bacc, mybir  # noqa: E402
from concourse.bass_utils import run_bass_kernel_spmd  # noqa: E402

N, F = 2048, 128
HEADS, OUT_DIM = 8, 8
ALPHA = 0.2
NCORES = 8
ROWS = N // NCORES          # 256 own rows per core
BLOCKS = ROWS // 16         # 16 blocks of 16 rows
FP = mybir.dt.float32
FR = mybir.dt.float32r
AOP = mybir.AluOpType

# score dtype: bf16 halves DVE time on the big elementwise passes
SCORE_BF16 = os.environ.get("GAT_SCORE_BF16", "1") == "1"
SDT = mybir.dt.bfloat16 if SCORE_BF16 else FP


def _mm_chunks(nc, out_ps, lhsT, rhs, free, maxn):
    """matmul out = lhsT.T @ rhs with the moving operand split into <=maxn cols."""
    for c0 in range(0, free, maxn):
        c1 = min(c0 + maxn, free)
        nc.tensor.matmul(out_ps[:, c0:c1], lhsT, rhs[:, c0:c1], start=True, stop=True)


def build_program():
    nc = bacc.Bacc("TRN2", debug=False)

    xt_d = nc.dram_tensor("XT", [F, N], FP, kind="ExternalInput")
    xto_d = nc.dram_tensor("XTo", [F, ROWS], FP, kind="ExternalInput")
    w_d = nc.dram_tensor("Wmat", [F, 64], FP, kind="ExternalInput")
    wa1_d = nc.dram_tensor("WA1", [F, HEADS], FP, kind="ExternalInput")
    wa2_d = nc.dram_tensor("WA2", [F, HEADS], FP, kind="ExternalInput")
    mask_d = nc.dram_tensor("MASKB", [ROWS, N], SDT, kind="ExternalInput")
    repl16_d = nc.dram_tensor("REPL16", [128, 128], SDT, kind="ExternalInput")
    bd_d = nc.dram_tensor("BD_MASK", [128, 128], FP, kind="ExternalInput")
    id_d = nc.dram_tensor("IDENT", [128, 128], FP, kind="ExternalInput")
    out_d = nc.dram_tensor("OUTC", [ROWS * 8, N], FP, kind="ExternalOutput")

    MMF = 512   # fp32 moving-operand free-dim limit
    MMB = 512   # PSUM fp32 bank limit applies to output cols

    with ExitStack() as ctx:
        tc = ctx.enter_context(tile.TileContext(nc))
        # persistent SBUF state
        per = ctx.enter_context(tc.tile_pool(name="persist", bufs=1))
        e2_rep = per.tile([128, N], SDT, tag="e2")
        s2t_rep = per.tile([128, N], SDT, tag="s2t")
        h_nmaj = [per.tile([128, 64], FP, tag=f"hn{i}", name=f"hn{i}") for i in range(2)]
        r_nmaj = [per.tile([128, HEADS], FP, tag=f"rn{i}", name=f"rn{i}") for i in range(2)]
        r_all = per.tile([128, BLOCKS], FP, tag="rall")
        h_all = per.tile([128, BLOCKS * HEADS], FP, tag="hall")
        bd_mask = per.tile([128, 128], FP, tag="bd")
        alpha_v = per.tile([128, 1], FP, tag="al")
        nc.vector.memset(alpha_v[:], ALPHA)
        nc.gpsimd.dma_start(bd_mask[:], bd_d.ap())

        # ---------------- preprocessing ----------------
        with tc.tile_pool(name="pre", bufs=1) as pre, \
             tc.tile_pool(name="pre_ps", bufs=1, space="PSUM") as pre_ps:
            ident = pre.tile([128, 128], FP)
            nc.gpsimd.dma_start(ident[:], id_d.ap())
            wmat = pre.tile([F, 64], FP)
            nc.gpsimd.dma_start(wmat[:], w_d.ap())
            xt = pre.tile([F, N], FP)
            nc.gpsimd.dma_start(xt[:], xt_d.ap())
            xto = pre.tile([F, ROWS], FP)
            nc.gpsimd.dma_start(xto[:], xto_d.ap())
            wa1 = pre.tile([F, HEADS], FP)
            nc.gpsimd.dma_start(wa1[:], wa1_d.ap())
            wa2 = pre.tile([F, HEADS], FP)
            nc.gpsimd.dma_start(wa2[:], wa2_d.ap())

            # s2T = (X @ W @ a2blk)^T directly: [8, N]
            s2t_ps = pre_ps.tile([HEADS, N], FP, tag="big")
            _mm_chunks(nc, s2t_ps, wa2[:], xt[:], N, MMF)
            nc.scalar.activation(e2_rep[:HEADS, :], s2t_ps[:],
                                 mybir.ActivationFunctionType.Exp)
            nc.scalar.copy(s2t_rep[:HEADS, :], s2t_ps[:])
            # log-doubling replication: 8 -> 16 -> 32 -> 64 -> 128 partitions
            for g in (8, 16, 32, 64):
                nc.gpsimd.dma_start(e2_rep[g:2 * g, :], e2_rep[:g, :])
                nc.gpsimd.dma_start(s2t_rep[g:2 * g, :], s2t_rep[:g, :])

            # own rows: hTo (for h_nmaj) and s1o
            hto_ps = pre_ps.tile([64, ROWS], FP, tag="small")
            _mm_chunks(nc, hto_ps, wmat[:], xto[:], ROWS, MMF)
            hto = pre.tile([64, ROWS], FP)
            nc.scalar.copy(hto[:], hto_ps[:])
            s1o_ps = pre_ps.tile([HEADS, ROWS], FP, tag="small2")
            _mm_chunks(nc, s1o_ps, wa1[:], xto[:], ROWS, MMF)
            s1o = pre.tile([HEADS, ROWS], FP)
            nc.scalar.copy(s1o[:], s1o_ps[:])
            for i in range(2):
                tp = pre_ps.tile([128, HEADS], FP, tag="tiny")
                nc.tensor.transpose(tp[:], s1o[:, i * 128:(i + 1) * 128],
                                    ident[:HEADS, :HEADS])
                nc.scalar.activation(r_nmaj[i][:], tp[:],
                                     mybir.ActivationFunctionType.Copy, scale=-0.8)
                tp2 = pre_ps.tile([128, 64], FP, tag="tiny")
                nc.tensor.transpose(tp2[:], hto[:, i * 128:(i + 1) * 128],
                                    ident[:64, :64])
                nc.scalar.copy(h_nmaj[i][:], tp2[:])
            for b in range(BLOCKS):
                half, row = divmod(b * 16, 128)
                nc.gpsimd.dma_start(r_all[:, b:b + 1], r_nmaj[half][row:row + 16, :])
                nc.gpsimd.dma_start(h_all[:, b * HEADS:(b + 1) * HEADS],
                                    h_nmaj[half][row:row + 16, :])

        # ---------------- main loop over 16-row blocks ----------------
        repl16 = per.tile([128, 128], SDT, tag="repl16")
        nc.gpsimd.dma_start(repl16[:], repl16_d.ap())
        # manual double-buffered padded mask tiles (rows 16+ stay zero)
        maskp = [per.tile([128, N], SDT, tag=f"maskp{i}", name=f"maskp{i}")
                 for i in range(2)]
        nc.vector.memset(maskp[0][:], 0.0)
        nc.vector.memset(maskp[1][:], 0.0)

        sb = ctx.enter_context(tc.tile_pool(name="blk", bufs=2))
        sb_q = ctx.enter_context(tc.tile_pool(name="blkq", bufs=3))
        ps_m = ctx.enter_context(tc.tile_pool(name="psm", bufs=1, space="PSUM"))
        ps_y = ctx.enter_context(tc.tile_pool(name="psy", bufs=1, space="PSUM"))

        for b in range(BLOCKS):
            # mask rows -> PE-replicated [128, N] in PSUM (p = n_local*8 + x)
            maskb = maskp[b % 2]
            nc.sync.dma_start(maskb[:16, :], mask_d.ap()[b * 16:(b + 1) * 16, :])
            m_rep = ps_m.tile([128, N], FP, tag="mrep")
            _mm_chunks(nc, m_rep, repl16[:], maskb[:], N, MMB if SCORE_BF16 else MMF)

            rb = r_all[:, b:b + 1]

            # u = r*E2b = exp(0.2*s2 - 0.8*s1)  on ACT; v = max(E2, u) on DVE
            u = sb.tile([128, N], SDT, tag="u")
            nc.scalar.activation(u[:], s2t_rep[:], mybir.ActivationFunctionType.Exp,
                                 bias=rb, scale=ALPHA)
            v = sb.tile([128, N], SDT, tag="v")
            nc.vector.tensor_tensor(v[:], u[:], e2_rep[:], AOP.max)

            # q = v * mask ; Dq = sum_j q   (one fused DVE op)
            q = sb_q.tile([128, N], FR, tag="q")
            dq = sb.tile([128, 1], FP, tag="dq")
            nc.vector.scalar_tensor_tensor(q[:], v[:], 1.0, m_rep[:],
                                           op0=AOP.mult, op1=AOP.mult, accum_out=dq[:])

            # W_blk[p=nh, f=n'd] = h_own[n,h*8+d]/Dq[nh] * blockdiag(n==n')
            rdq = sb.tile([128, 1], FP, tag="rdq")
            nc.vector.reciprocal(rdq[:], dq[:])
            hb = h_all[:, b * HEADS:(b + 1) * HEADS]
            wblk = sb.tile([128, 128], FR, tag="wblk")
            nc.vector.scalar_tensor_tensor(
                wblk[:].rearrange("p (o e) -> p o e", o=16),
                hb.rearrange("p (o e) -> p o e", o=1).broadcast_to([128, 16, HEADS]),
                rdq[:],
                bd_mask[:].rearrange("p (o e) -> p o e", o=16),
                op0=AOP.mult, op1=AOP.mult)

            # y[p=nd, j] = sum_h W_blk[nh, nd] q[nh, j] ; out = lrelu(y)
            y_ps = ps_y.tile([128, N], FP, tag="y")
            _mm_chunks(nc, y_ps, wblk[:], q[:], N, MMF)
            out_sb = sb_q.tile([128, N], FP, tag="out")
            nc.scalar.activation(out_sb[:], y_ps[:],
                                 mybir.ActivationFunctionType.Prelu, alpha=alpha_v[:])
            nc.sync.dma_start(out_d.ap()[b * 128:(b + 1) * 128, :N // 2],
                              out_sb[:, :N // 2])
            nc.sync.dma_start(out_d.ap()[b * 128:(b + 1) * 128, N // 2:],
                              out_sb[:, N // 2:])

    nc.compile()
    return nc


_NC_CACHE = None


def _get_program():
    global _NC_CACHE
    if _NC_CACHE is None:
        _NC_CACHE = build_program()
    return _NC_CACHE


def _host_inputs(X, A, W, attn_kernel):
    import ml_dtypes
    mdt = ml_dtypes.bfloat16 if SCORE_BF16 else np.float32

    XT = np.ascontiguousarray(X.T).astype(np.float32)
    a1 = attn_kernel[:OUT_DIM, 0].astype(np.float32)
    a2 = attn_kernel[OUT_DIM:, 0].astype(np.float32)
    # fold the tiny attention vectors into W: s1 = X @ (W . a1-per-head)
    Wf = W.astype(np.float32).reshape(F, HEADS, OUT_DIM)
    WA1 = np.ascontiguousarray(Wf @ a1)   # [F, HEADS]
    WA2 = np.ascontiguousarray(Wf @ a2)   # [F, HEADS]
    REPL16 = np.zeros((128, 128), np.float32)
    for nl in range(16):
        REPL16[nl, nl * 8:(nl + 1) * 8] = 1.0
    BD = np.zeros((128, 128), np.float32)
    for nl in range(16):
        BD[nl * 8:(nl + 1) * 8, nl * 8:(nl + 1) * 8] = 1.0
    IDENT = np.eye(128, dtype=np.float32)

    Af = (A > 0).astype(np.float32)
    in_maps = []
    for c in range(NCORES):
        n0 = c * ROWS
        in_maps.append({
            "XT": XT,
            "XTo": np.ascontiguousarray(XT[:, n0:n0 + ROWS]),
            "Wmat": W.astype(np.float32),
            "WA1": WA1, "WA2": WA2,
            "MASKB": Af[n0:n0 + ROWS].astype(mdt),
            "REPL16": REPL16.astype(mdt),
            "BD_MASK": BD, "IDENT": IDENT,
        })
    return in_maps


def kernel(X, A, W, attn_kernel, _want_timing=False):
    X = np.asarray(X)
    A = np.asarray(A)
    W = np.asarray(W)
    attn_kernel = np.asarray(attn_kernel)
    nc = _get_program()
    in_maps = _host_inputs(X, A, W, attn_kernel)
    res = run_bass_kernel_spmd(nc, in_maps, core_ids=list(range(NCORES)),
                               trace=_want_timing)
    # device rows are (block, n_local, d) x (j); reference wants (n, j, d)
    parts = []
    for c in range(NCORES):
        oc = res.results[c]["OUTC"]                        # [2048, 2048]
        oc = oc.reshape(BLOCKS, 16, OUT_DIM, N)            # [b, nl, d, j]
        oc = oc.transpose(0, 1, 3, 2).reshape(-1, OUT_DIM * HEADS)
        parts.append(oc)
    out = np.concatenate(parts, axis=0)
    if _want_timing:
        return out, res
    return out
